# revision 1
# baseline (speedup 1.0000x reference)
"""Fused OT-DTW l2 cost-matrix kernel for Trainium2 (8 NeuronCores, SPMD).

mat_cost[i,j] = sum_{t,p,d} pi[cl(i)][t,p] * (X[i,t,d] - Y[j,p,d])^2
             = C1[i] + C2[cl(i), j] - 2 * C3[i,j]

with C3[i,j] = sum_{p,d} XP[i,p,d] * Y[j,p,d],  XP[i] = X[i].T @ pi[cl(i)]
(as [D,TP], i.e. XPT). The device computes the heavy parts (XP: ~69 GFLOP,
C3: ~137 GFLOP) in bf16 on 8 cores, data-parallel over rows of X. The tiny
rank-1 corrections C1/C2 (<0.2% of FLOPs) are applied on the host.

Sharding: core k takes X rows [128k, 128k+128), Y/pi replicated. Host
pre-permutes all operands so every device DMA is contiguous:
  xs  [i, t, c, d]   (c = t-chunk of 128) bf16, per core
  pig [i, t, c, p]   pi[classe[i]] pre-gathered, bf16, per core
  yt  [d, p, j]      Y transposed, bf16, shared
Per core: stage A does, for each i, XPT_i[d,p] = X[i].T @ pi_c (4 matmuls,
K=t tiles of 128, N=512), cast to bf16 into a resident SBUF tile
xpt[d, p, i]. Stage B accumulates C3[i, j] over 512 p-tiles: lhsT =
xpt[:, p, :] (d-major), rhs = yt tile [d, 4p, 1024j] streamed from HBM.
"""

import os
import sys
import types

import numpy as np
import ml_dtypes

NX, NY, T, TP, D, C = 1024, 1024, 512, 512, 128, 8
N_CORES = 8
NL = NX // N_CORES          # 128 rows per core
TC = T // 128               # 4 t-chunks
PG = 4                      # p-tiles per Y DMA
BF16 = ml_dtypes.bfloat16


def _ensure_axon_hooks():
    """concourse.bass_utils imports antenv.axon_hooks when tracing under
    axon; some images lack that submodule. Provide it, and register the
    NTFF profile hook if the boot path didn't."""
    try:
        import antenv
    except ImportError:
        return
    try:
        from antenv import axon_hooks  # noqa: F401
    except ImportError:
        mod = types.ModuleType("antenv.axon_hooks")
        mod._hook = None

        def _set(h):
            mod._hook = h

        def _get():
            return mod._hook

        mod.set_axon_ntff_profile_hook = _set
        mod.get_axon_ntff_profile_hook = _get
        sys.modules["antenv.axon_hooks"] = mod
        antenv.axon_hooks = mod
    from antenv.axon_hooks import (
        get_axon_ntff_profile_hook,
        set_axon_ntff_profile_hook,
    )

    if get_axon_ntff_profile_hook() is None:
        try:
            from trn_agent_boot.trn_boot import _ntff_profile_via_ctypes

            hook = _ntff_profile_via_ctypes("/opt/axon/libaxon_pjrt.so")
            if hook is not None:
                set_axon_ntff_profile_hook(hook)
        except Exception:
            pass


_ensure_axon_hooks()

import concourse.bass as bass  # noqa: E402
import concourse.tile as tile  # noqa: E402
from concourse import bacc, mybir  # noqa: E402
from concourse.bass_utils import run_bass_kernel_spmd  # noqa: E402

_PROGRAM_CACHE = {}
LAST_RUN = None  # BassKernelResults of the most recent kernel() call


def _build_program():
    if "nc" in _PROGRAM_CACHE:
        return _PROGRAM_CACHE["nc"]
    bf = mybir.dt.bfloat16
    f32 = mybir.dt.float32
    nc = bacc.Bacc("TRN2", target_bir_lowering=False, debug=False,
                   num_devices=N_CORES)
    xs = nc.dram_tensor("xs", [NL, 128, TC, D], bf, kind="ExternalInput").ap()
    pig = nc.dram_tensor("pig", [NL, 128, TC, TP], bf, kind="ExternalInput").ap()
    yt = nc.dram_tensor("yt", [D, TP, NY], bf, kind="ExternalInput").ap()
    c3 = nc.dram_tensor("c3", [NL, NY], f32, kind="ExternalOutput").ap()

    with tile.TileContext(nc) as tc:
        with (
            tc.tile_pool(name="xpt", bufs=1) as xpt_pool,
            tc.tile_pool(name="xin", bufs=3) as xin_pool,
            tc.tile_pool(name="pigin", bufs=3) as pig_pool,
            tc.tile_pool(name="yin", bufs=4) as y_pool,
            tc.tile_pool(name="outsb", bufs=1) as out_pool,
            tc.tile_pool(name="psA", bufs=2, space="PSUM") as psA_pool,
            tc.tile_pool(name="psB", bufs=1, space="PSUM") as psB_pool,
        ):
            # Resident transposed XP for all local rows: [d, p, i] bf16.
            xpt = xpt_pool.tile([D, TP, NL], bf)

            # ---- Stage A: XPT_i = X[i].T @ pi[cl(i)] ----
            for i in range(NL):
                xt = xin_pool.tile([128, TC, D], bf)
                nc.sync.dma_start(xt[:], xs[i])
                pt = pig_pool.tile([128, TC, TP], bf)
                nc.sync.dma_start(pt[:], pig[i])
                acc = psA_pool.tile([D, TP], f32)
                for c in range(TC):
                    nc.tensor.matmul(
                        acc[:], xt[:, c, :], pt[:, c, :],
                        start=(c == 0), stop=(c == TC - 1),
                    )
                nc.vector.tensor_copy(xpt[:, :, i], acc[:])

            # ---- Stage B: C3[i, j] = sum_p XPT[:, p, i].T @ YT[:, p, j] ----
            accj0 = psB_pool.tile([NL, 512], f32)
            accj1 = psB_pool.tile([NL, 512], f32)
            for g in range(TP // PG):
                ytile = y_pool.tile([D, PG, NY], bf)
                nc.sync.dma_start(ytile[:], yt[:, g * PG:(g + 1) * PG, :])
                for s in range(PG):
                    p = g * PG + s
                    st, sp = (p == 0), (p == TP - 1)
                    nc.tensor.matmul(accj0[:], xpt[:, p, :], ytile[:, s, 0:512],
                                     start=st, stop=sp)
                    nc.tensor.matmul(accj1[:], xpt[:, p, :], ytile[:, s, 512:1024],
                                     start=st, stop=sp)

            out_sb = out_pool.tile([NL, NY], f32)
            nc.vector.tensor_copy(out_sb[:, 0:512], accj0[:])
            nc.vector.tensor_copy(out_sb[:, 512:1024], accj1[:])
            nc.sync.dma_start(c3[:], out_sb[:])

    nc.compile()
    _PROGRAM_CACHE["nc"] = nc
    return nc


def kernel(X, Y, pi, classe):
    global LAST_RUN
    assert X.shape == (NX, T, D) and Y.shape == (NY, TP, D)
    assert pi.shape == (C, T, TP) and classe.shape == (NX,)
    X = np.asarray(X, dtype=np.float32)
    Y = np.asarray(Y, dtype=np.float32)
    pi = np.asarray(pi, dtype=np.float32)
    classe = np.asarray(classe)

    nc = _build_program()

    # Host-side sharding + layout prep (all-contiguous device DMAs).
    pi_bf = pi.astype(BF16)
    # yt[d, p, j] = Y[j, p, d]
    yt = np.ascontiguousarray(Y.transpose(2, 1, 0).astype(BF16))
    in_maps = []
    for k in range(N_CORES):
        rows = slice(k * NL, (k + 1) * NL)
        xk = X[rows].astype(BF16)                      # [NL, T, D]
        xk = np.ascontiguousarray(
            xk.reshape(NL, TC, 128, D).transpose(0, 2, 1, 3))
        pk = pi_bf[classe[rows]]                       # [NL, T, TP]
        pk = np.ascontiguousarray(
            pk.reshape(NL, TC, 128, TP).transpose(0, 2, 1, 3))
        in_maps.append({"xs": xk, "pig": pk, "yt": yt})

    trace = bool(os.environ.get("BASS_TRACE"))
    LAST_RUN = run_bass_kernel_spmd(nc, in_maps, list(range(N_CORES)),
                                    trace=trace)
    C3 = np.concatenate([LAST_RUN.results[k]["c3"] for k in range(N_CORES)],
                        axis=0)

    # Host epilogue: rank-1 corrections (0.15% of FLOPs).
    row_c = pi.sum(-1)                                 # [C, T]
    col_c = pi.sum(1)                                  # [C, TP]
    SX = np.einsum("itd,itd->it", X, X)                # [NX, T]
    SY = np.einsum("jpd,jpd->jp", Y, Y)                # [NY, TP]
    C1 = np.einsum("it,it->i", SX, row_c[classe])      # [NX]
    C2 = col_c @ SY.T                                  # [C, NY]
    return (C1[:, None] + C2[classe] - 2.0 * C3).astype(np.float32)


# revision 4
# speedup vs baseline: 1.4484x; 1.4484x over previous
"""Fused OT-DTW l2 cost-matrix kernel for Trainium2 (8 NeuronCores, SPMD).

mat_cost[i,j] = sum_{t,p,d} pi[cl(i)][t,p] * (X[i,t,d] - Y[j,p,d])^2
             = C1[i] + C2[cl(i), j] - 2 * C3[i,j]

with C3[i,j] = sum_{p,d} XP[i,p,d] * Y[j,p,d],  XP[i] = X[i].T @ pi[cl(i)]
(as [D,TP], i.e. XPT). The device computes the heavy parts (XP: ~69 GFLOP,
C3: ~137 GFLOP) in bf16 on 8 cores, data-parallel over rows of X. The tiny
rank-1 corrections C1/C2 (<0.2% of FLOPs) are applied on the host.

Sharding: core k takes X rows [128k, 128k+128), Y/pi replicated. Host
pre-permutes all operands so every device DMA is contiguous:
  xs  [i, t, c, d]   (c = t-chunk of 128) bf16, per core
  pig [i, t, c, p]   pi[classe[i]] pre-gathered, bf16, per core
  yt  [d, p, j]      Y transposed, bf16, shared
Per core: stage A does, for each i, XPT_i[d,p] = X[i].T @ pi_c (4 matmuls,
K=t tiles of 128, N=512), cast to bf16 into a resident SBUF tile
xpt[d, p, i]. Stage B accumulates C3[i, j] over 512 p-tiles: lhsT =
xpt[:, p, :] (d-major), rhs = yt tile [d, 4p, 1024j] streamed from HBM.
"""

import os
import sys
import types

import numpy as np
import ml_dtypes

NX, NY, T, TP, D, C = 1024, 1024, 512, 512, 128, 8
N_CORES = 8
NL = NX // N_CORES          # 128 rows per core
TC = T // 128               # 4 t-chunks
PG = 8                      # p-tiles per Y DMA
BF16 = ml_dtypes.bfloat16
F8 = ml_dtypes.float8_e4m3fn


def _ensure_axon_hooks():
    """concourse.bass_utils imports antenv.axon_hooks when tracing under
    axon; some images lack that submodule. Provide it, and register the
    NTFF profile hook if the boot path didn't."""
    try:
        import antenv
    except ImportError:
        return
    try:
        from antenv import axon_hooks  # noqa: F401
    except ImportError:
        mod = types.ModuleType("antenv.axon_hooks")
        mod._hook = None

        def _set(h):
            mod._hook = h

        def _get():
            return mod._hook

        mod.set_axon_ntff_profile_hook = _set
        mod.get_axon_ntff_profile_hook = _get
        sys.modules["antenv.axon_hooks"] = mod
        antenv.axon_hooks = mod
    from antenv.axon_hooks import (
        get_axon_ntff_profile_hook,
        set_axon_ntff_profile_hook,
    )

    if get_axon_ntff_profile_hook() is None:
        try:
            from trn_agent_boot.trn_boot import _ntff_profile_via_ctypes

            hook = _ntff_profile_via_ctypes("/opt/axon/libaxon_pjrt.so")
            if hook is not None:
                set_axon_ntff_profile_hook(hook)
        except Exception:
            pass


_ensure_axon_hooks()

import concourse.bass as bass  # noqa: E402
import concourse.tile as tile  # noqa: E402
from concourse import bacc, mybir  # noqa: E402
from concourse.bass_utils import run_bass_kernel_spmd  # noqa: E402

_PROGRAM_CACHE = {}
LAST_RUN = None  # BassKernelResults of the most recent kernel() call


def _build_program():
    if "nc" in _PROGRAM_CACHE:
        return _PROGRAM_CACHE["nc"]
    bf = mybir.dt.bfloat16
    f8 = mybir.dt.float8e4
    f32 = mybir.dt.float32
    nc = bacc.Bacc("TRN2", target_bir_lowering=False, debug=False,
                   num_devices=N_CORES)
    xs = nc.dram_tensor("xs", [NL, 128, TC, D], bf, kind="ExternalInput").ap()
    pig = nc.dram_tensor("pig", [NL, 128, TC, TP], f8, kind="ExternalInput").ap()
    yt = nc.dram_tensor("yt", [D, TP, NY], f8, kind="ExternalInput").ap()
    c3 = nc.dram_tensor("c3", [NL, NY], f32, kind="ExternalOutput").ap()

    with tile.TileContext(nc) as tc:
        with (
            tc.tile_pool(name="xpt", bufs=1) as xpt_pool,
            tc.tile_pool(name="xin", bufs=3) as xin_pool,
            tc.tile_pool(name="pigin", bufs=3) as pig_pool,
            tc.tile_pool(name="yin", bufs=4) as y_pool,
            tc.tile_pool(name="outsb", bufs=1) as out_pool,
            tc.tile_pool(name="psA", bufs=2, space="PSUM") as psA_pool,
            tc.tile_pool(name="psB", bufs=1, space="PSUM") as psB_pool,
        ):
            # Resident transposed XP for all local rows: [d, i, p] bf16
            # (i-major so the per-i PSUM->SBUF cast writes contiguously).
            xpt = xpt_pool.tile([D, NL, TP], bf)

            # ---- Stage A: XPT_i = X[i].T @ pi[cl(i)] ----
            for i in range(NL):
                xt = xin_pool.tile([128, TC, D], bf)
                nc.sync.dma_start(xt[:], xs[i])
                pt = pig_pool.tile([128, TC, TP], f8)
                nc.sync.dma_start(pt[:], pig[i])
                acc = psA_pool.tile([D, TP], f32)
                for c in range(TC):
                    nc.tensor.matmul(
                        acc[:], xt[:, c, :], pt[:, c, :],
                        start=(c == 0), stop=(c == TC - 1),
                    )
                nc.vector.tensor_copy(xpt[:, i, :], acc[:])

            # ---- Stage B: C3[i, j] = sum_p XPT[:, :, p].T @ YT[:, p, j] ----
            accj0 = psB_pool.tile([NL, 512], f32)
            accj1 = psB_pool.tile([NL, 512], f32)
            for g in range(TP // PG):
                ytile = y_pool.tile([D, PG, NY], f8)
                nc.sync.dma_start(ytile[:], yt[:, g * PG:(g + 1) * PG, :])
                for s in range(PG):
                    p = g * PG + s
                    st, sp = (p == 0), (p == TP - 1)
                    nc.tensor.matmul(accj0[:], xpt[:, :, p], ytile[:, s, 0:512],
                                     start=st, stop=sp)
                    nc.tensor.matmul(accj1[:], xpt[:, :, p], ytile[:, s, 512:1024],
                                     start=st, stop=sp)

            out_sb = out_pool.tile([NL, NY], f32)
            nc.vector.tensor_copy(out_sb[:, 0:512], accj0[:])
            nc.vector.tensor_copy(out_sb[:, 512:1024], accj1[:])
            nc.sync.dma_start(c3[:], out_sb[:])

    nc.compile()
    _PROGRAM_CACHE["nc"] = nc
    return nc


def kernel(X, Y, pi, classe):
    global LAST_RUN
    assert X.shape == (NX, T, D) and Y.shape == (NY, TP, D)
    assert pi.shape == (C, T, TP) and classe.shape == (NX,)
    X = np.asarray(X, dtype=np.float32)
    Y = np.asarray(Y, dtype=np.float32)
    pi = np.asarray(pi, dtype=np.float32)
    classe = np.asarray(classe)

    nc = _build_program()

    # Host-side sharding + layout prep (all-contiguous device DMAs).
    pi_f8 = pi.astype(F8)                              # 0/1 -> exact in fp8
    # yt[d, p, j] = Y[j, p, d]
    yt = np.ascontiguousarray(Y.transpose(2, 1, 0).astype(F8))
    in_maps = []
    for k in range(N_CORES):
        rows = slice(k * NL, (k + 1) * NL)
        xk = X[rows].astype(BF16)                      # [NL, T, D]
        xk = np.ascontiguousarray(
            xk.reshape(NL, TC, 128, D).transpose(0, 2, 1, 3))
        pk = pi_f8[classe[rows]]                       # [NL, T, TP]
        pk = np.ascontiguousarray(
            pk.reshape(NL, TC, 128, TP).transpose(0, 2, 1, 3))
        in_maps.append({"xs": xk, "pig": pk, "yt": yt})

    trace = bool(os.environ.get("BASS_TRACE"))
    LAST_RUN = run_bass_kernel_spmd(nc, in_maps, list(range(N_CORES)),
                                    trace=trace)
    C3 = np.concatenate([LAST_RUN.results[k]["c3"] for k in range(N_CORES)],
                        axis=0)

    # Host epilogue: rank-1 corrections (0.15% of FLOPs).
    row_c = pi.sum(-1)                                 # [C, T]
    col_c = pi.sum(1)                                  # [C, TP]
    SX = np.einsum("itd,itd->it", X, X)                # [NX, T]
    SY = np.einsum("jpd,jpd->jp", Y, Y)                # [NY, TP]
    C1 = np.einsum("it,it->i", SX, row_c[classe])      # [NX]
    C2 = col_c @ SY.T                                  # [C, NY]
    return (C1[:, None] + C2[classe] - 2.0 * C3).astype(np.float32)


# revision 6
# speedup vs baseline: 2.0526x; 1.4172x over previous
"""Fused OT-DTW l2 cost-matrix kernel for Trainium2 (8 NeuronCores, SPMD).

mat_cost[i,j] = sum_{t,p,d} pi[cl(i)][t,p] * (X[i,t,d] - Y[j,p,d])^2
             = C1[i] + C2[cl(i), j] - 2 * C3[i,j]

with C3[i,j] = sum_{p,d} XP[i,p,d] * Y[j,p,d],  XP[i] = X[i].T @ pi[cl(i)]
(as [D,TP], i.e. XPT). The device computes the heavy parts (XP: ~69 GFLOP,
C3: ~137 GFLOP) in bf16 on 8 cores, data-parallel over rows of X. The tiny
rank-1 corrections C1/C2 (<0.2% of FLOPs) are applied on the host.

Sharding: core k takes X rows [128k, 128k+128), Y/pi replicated. Host
pre-permutes all operands so every device DMA is contiguous:
  xs  [i, t, c, d]   (c = t-chunk of 128) bf16, per core
  pig [i, t, c, p]   pi[classe[i]] pre-gathered, bf16, per core
  yt  [d, p, j]      Y transposed, bf16, shared
Per core: stage A does, for each i, XPT_i[d,p] = X[i].T @ pi_c (4 matmuls,
K=t tiles of 128, N=512), cast to bf16 into a resident SBUF tile
xpt[d, p, i]. Stage B accumulates C3[i, j] over 512 p-tiles: lhsT =
xpt[:, p, :] (d-major), rhs = yt tile [d, 4p, 1024j] streamed from HBM.
"""

import os
import sys
import types

import numpy as np
import ml_dtypes

NX, NY, T, TP, D, C = 1024, 1024, 512, 512, 128, 8
N_CORES = 8
NL = NX // N_CORES          # 128 rows per core
TC = T // 128               # 4 t-chunks
PG = 8                      # p-tiles per Y DMA
BF16 = ml_dtypes.bfloat16
F8 = ml_dtypes.float8_e4m3fn


def _ensure_axon_hooks():
    """concourse.bass_utils imports antenv.axon_hooks when tracing under
    axon; some images lack that submodule. Provide it, and register the
    NTFF profile hook if the boot path didn't."""
    try:
        import antenv
    except ImportError:
        return
    try:
        from antenv import axon_hooks  # noqa: F401
    except ImportError:
        mod = types.ModuleType("antenv.axon_hooks")
        mod._hook = None

        def _set(h):
            mod._hook = h

        def _get():
            return mod._hook

        mod.set_axon_ntff_profile_hook = _set
        mod.get_axon_ntff_profile_hook = _get
        sys.modules["antenv.axon_hooks"] = mod
        antenv.axon_hooks = mod
    from antenv.axon_hooks import (
        get_axon_ntff_profile_hook,
        set_axon_ntff_profile_hook,
    )

    if get_axon_ntff_profile_hook() is None:
        try:
            from trn_agent_boot.trn_boot import _ntff_profile_via_ctypes

            hook = _ntff_profile_via_ctypes("/opt/axon/libaxon_pjrt.so")
            if hook is not None:
                set_axon_ntff_profile_hook(hook)
        except Exception:
            pass


_ensure_axon_hooks()

import concourse.bass as bass  # noqa: E402
import concourse.tile as tile  # noqa: E402
from concourse import bacc, mybir  # noqa: E402
from concourse.bass_utils import run_bass_kernel_spmd  # noqa: E402

_PROGRAM_CACHE = {}
LAST_RUN = None  # BassKernelResults of the most recent kernel() call


def _build_program():
    if "nc" in _PROGRAM_CACHE:
        return _PROGRAM_CACHE["nc"]
    f8 = mybir.dt.float8e4
    f32 = mybir.dt.float32
    DR = mybir.MatmulPerfMode.DoubleRow
    nc = bacc.Bacc("TRN2", target_bir_lowering=False, debug=False,
                   num_devices=N_CORES)
    xs = nc.dram_tensor("xs", [NL, 128, TC, D], f8, kind="ExternalInput").ap()
    pig = nc.dram_tensor("pig", [NL, 128, TC, TP], f8, kind="ExternalInput").ap()
    yt = nc.dram_tensor("yt", [D, TP, NY], f8, kind="ExternalInput").ap()
    c3 = nc.dram_tensor("c3", [NL, NY], f32, kind="ExternalOutput").ap()

    with tile.TileContext(nc) as tc:
        with (
            tc.tile_pool(name="xpt", bufs=1) as xpt_pool,
            tc.tile_pool(name="xin", bufs=4) as xin_pool,
            tc.tile_pool(name="pigin", bufs=4) as pig_pool,
            tc.tile_pool(name="yin", bufs=10) as y_pool,
            tc.tile_pool(name="stg", bufs=3) as stg_pool,
            tc.tile_pool(name="outsb", bufs=1) as out_pool,
            tc.tile_pool(name="psA", bufs=4, space="PSUM") as psA_pool,
            tc.tile_pool(name="psB", bufs=1, space="PSUM") as psB_pool,
        ):
            # Resident transposed XP for all local rows: [d, p, i] fp8
            # (p-major pairs for DoubleRow lhsT interleave).
            xpt = xpt_pool.tile([D, TP, NL], f8)

            # ---- Stage A: XPT_i = X[i].T @ pi[cl(i)], fp8 DoubleRow over t ----
            for i in range(NL):
                xt = xin_pool.tile([128, TC, D], f8)
                nc.sync.dma_start(xt[:], xs[i])
                pt = pig_pool.tile([128, TC, TP], f8)
                nc.sync.dma_start(pt[:], pig[i])
                acc = psA_pool.tile([D, TP], f32)
                for c in range(TC // 2):
                    nc.tensor.matmul(
                        acc[:], xt[:, 2 * c:2 * c + 2, :], pt[:, 2 * c:2 * c + 2, :],
                        start=(c == 0), stop=(c == TC // 2 - 1),
                        perf_mode=DR,
                    )
                # Corner-turn cast psum[d, p] -> xpt[d, :, i] (1B @ 128B stride),
                # round-robined over DVE / ACT / (DVE contig + GpSimd strided).
                m = i % 3
                if m == 0:
                    nc.vector.tensor_copy(xpt[:, :, i], acc[:])
                elif m == 1:
                    nc.scalar.copy(xpt[:, :, i], acc[:])
                else:
                    stg = stg_pool.tile([D, TP], f8)
                    nc.vector.tensor_copy(stg[:], acc[:])
                    nc.gpsimd.tensor_copy(xpt[:, :, i], stg[:])

            # ---- Stage B: C3[i, j] = sum_p XPT[:, p, i] YT[:, p, j], DR pairs ----
            accj0 = psB_pool.tile([NL, 512], f32)
            accj1 = psB_pool.tile([NL, 512], f32)
            for g in range(TP // PG):
                ytile = y_pool.tile([D, PG, NY], f8)
                nc.sync.dma_start(ytile[:], yt[:, g * PG:(g + 1) * PG, :])
                for s in range(PG // 2):
                    p = g * PG + 2 * s
                    st, sp = (p == 0), (p == TP - 2)
                    lhsT = xpt[:, p:p + 2, :]
                    rhs = ytile[:, 2 * s:2 * s + 2, :]
                    nc.tensor.matmul(accj0[:], lhsT, rhs[:, :, 0:512],
                                     start=st, stop=sp, perf_mode=DR)
                    nc.tensor.matmul(accj1[:], lhsT, rhs[:, :, 512:1024],
                                     start=st, stop=sp, perf_mode=DR)

            out_sb = out_pool.tile([NL, NY], f32)
            nc.vector.tensor_copy(out_sb[:, 0:512], accj0[:])
            nc.vector.tensor_copy(out_sb[:, 512:1024], accj1[:])
            nc.sync.dma_start(c3[:], out_sb[:])

    nc.compile()
    _PROGRAM_CACHE["nc"] = nc
    return nc


def kernel(X, Y, pi, classe):
    global LAST_RUN
    assert X.shape == (NX, T, D) and Y.shape == (NY, TP, D)
    assert pi.shape == (C, T, TP) and classe.shape == (NX,)
    X = np.asarray(X, dtype=np.float32)
    Y = np.asarray(Y, dtype=np.float32)
    pi = np.asarray(pi, dtype=np.float32)
    classe = np.asarray(classe)

    nc = _build_program()

    # Host-side sharding + layout prep (all-contiguous device DMAs).
    pi_f8 = pi.astype(F8)                              # 0/1 -> exact in fp8
    # yt[d, p, j] = Y[j, p, d]
    yt = np.ascontiguousarray(Y.transpose(2, 1, 0).astype(F8))
    in_maps = []
    for k in range(N_CORES):
        rows = slice(k * NL, (k + 1) * NL)
        xk = X[rows].astype(F8)                        # [NL, T, D]
        xk = np.ascontiguousarray(
            xk.reshape(NL, TC, 128, D).transpose(0, 2, 1, 3))
        pk = pi_f8[classe[rows]]                       # [NL, T, TP]
        pk = np.ascontiguousarray(
            pk.reshape(NL, TC, 128, TP).transpose(0, 2, 1, 3))
        in_maps.append({"xs": xk, "pig": pk, "yt": yt})

    trace = bool(os.environ.get("BASS_TRACE"))
    LAST_RUN = run_bass_kernel_spmd(nc, in_maps, list(range(N_CORES)),
                                    trace=trace)
    C3 = np.concatenate([LAST_RUN.results[k]["c3"] for k in range(N_CORES)],
                        axis=0)

    # Host epilogue: rank-1 corrections (0.15% of FLOPs).
    row_c = pi.sum(-1)                                 # [C, T]
    col_c = pi.sum(1)                                  # [C, TP]
    SX = np.einsum("itd,itd->it", X, X)                # [NX, T]
    SY = np.einsum("jpd,jpd->jp", Y, Y)                # [NY, TP]
    C1 = np.einsum("it,it->i", SX, row_c[classe])      # [NX]
    C2 = col_c @ SY.T                                  # [C, NY]
    return (C1[:, None] + C2[classe] - 2.0 * C3).astype(np.float32)
